# revision 1
# baseline (speedup 1.0000x reference)
"""Trainium2 Bass kernel for nn_AttentionSortNet (sparse_attention).

Computes, per (batch*head) slice:
  sq = bucket-mean(q), sk = bucket-mean(k)          # (64, 64) each
  R  = sq @ sk.T * DIM**-0.5                        # (64, 64)
  r  = (log(relu(R)+eps) + gumbel(u)) / T
  8x log-domain Sinkhorn row/col normalization
  out = exp(r)

Strategy: shard the 32 bh slices across 8 cores (4 bh each, no
communication). On-core:

- q/k stream in over HWDGE at ~420 GB/s with buckets on the partition
  axis (8KB contiguous per partition per 1MiB chunk).
- Within-bucket summation is contiguous in-place halving adds, split
  across DVE and GpSimd so the combined reduce rate keeps up with DMA.
- Bucket-summary transposes and similarity matmuls run on PE.
- Sinkhorn runs in multiplicative form (P /= rowsum; P /= colsum) as
  two independent 2-bh problems: pair 0's iterations hide under pair
  1's loads. Row sums are DVE reduces with per-partition reciprocal
  multiply; column sums come pre-broadcast from one PE matmul against
  a block-diagonal ones matrix, applied with a tensor_tensor divide.

Built on bacc.Bacc (not raw Bass): its compile pass splits multi-sem
sync waits, which this walrus requires (one wait per instruction).
Constants are built before the TileContext behind an all-engine
barrier so tile instructions don't sync against them.
"""

import sys

for _p in ("/opt/trn_rl_repo",):
    if _p not in sys.path:
        sys.path.insert(0, _p)

import numpy as np

N_CORES = 8
BH = 32
B_PER = BH // N_CORES          # 4 bh per core
SEQ = 8192
D = 64
BUCKET_SIZE = 128
BUCKETS = SEQ // BUCKET_SIZE   # 64 buckets per bh
N_CHUNKS = 4                   # free-dim chunks per 128-bucket tile
CHUNK_W = BUCKET_SIZE // N_CHUNKS   # 32 within-bucket positions per chunk
CHUNK_F = CHUNK_W * D          # 2048 f32 per partition per chunk
EPS = 1e-6
TEMP = 0.7
SINKHORN_ITER = 8
# q/k are reduced to bucket *sums*; fold the two 1/128 mean factors and
# the DIM**-0.5 = 1/8 similarity scale into one constant applied at relu.
R_SCALE = 1.0 / (BUCKET_SIZE * BUCKET_SIZE * 8.0)

_NC_CACHE = None


def _build():
    import concourse.bacc as bacc
    import concourse.mybir as mybir
    import concourse.tile as tile
    from concourse.masks import make_identity
    from contextlib import ExitStack

    fp32 = mybir.dt.float32
    AX = mybir.AxisListType
    AF = mybir.ActivationFunctionType
    ALU = mybir.AluOpType

    from concourse.hw_specs import get_activation_tables
    import bass_rust as _bass_rust

    class _Bacc(bacc.Bacc):
        def insert_act_table_loads(self):
            # Restrict Ln/Exp to the combined natural_log_exp set so the
            # greedy chooser stops reloading ACT tables on every switch.
            has_act = any(
                isinstance(i, mybir.InstActivation)
                for b in self.main_func.blocks
                for i in b.instructions
            )
            if not has_act:
                return
            AF2 = mybir.ActivationFunctionType
            tables = []
            for name, funcs in get_activation_tables(self.m.arch).items():
                if name != "natural_log_exp_and_others":
                    funcs = {f for f in funcs if f not in (AF2.Ln, AF2.Exp)}
                tables.append((name, funcs))
            _bass_rust.insert_act_table_loads(self, tables)

    nc = _Bacc("TRN2", target_bir_lowering=False, debug=False)

    q = nc.dram_tensor("q", [B_PER, SEQ, D], fp32, kind="ExternalInput")
    k = nc.dram_tensor("k", [B_PER, SEQ, D], fp32, kind="ExternalInput")
    gu = nc.dram_tensor("gumbel_u", [B_PER, BUCKETS, BUCKETS], fp32,
                        kind="ExternalInput")
    out = nc.dram_tensor("out", [B_PER, BUCKETS, BUCKETS], fp32,
                         kind="ExternalOutput")

    # (b, s, d) -> (global bucket row, chunk, chunk payload)
    qv = q.ap().rearrange("b (bk c w) d -> (b bk) c (w d)", bk=BUCKETS, c=N_CHUNKS)
    kv = k.ap().rearrange("b (bk c w) d -> (b bk) c (w d)", bk=BUCKETS, c=N_CHUNKS)
    # bh = 2g + h laid out as partition p = 64h + i, free = (g, j)
    guv = gu.ap().rearrange("(g h) i j -> (h i) g j", h=2)
    outv = out.ap().rearrange("(g h) i j -> (h i) g j", h=2)

    with tile.TileContext(nc) as tc, ExitStack() as ctx:
        consts = ctx.enter_context(tc.tile_pool(name="consts", bufs=1))
        chunks = ctx.enter_context(tc.tile_pool(name="chunks", bufs=16))
        parts = ctx.enter_context(tc.tile_pool(name="parts", bufs=1))
        sums = ctx.enter_context(tc.tile_pool(name="sums", bufs=4))
        sbt = ctx.enter_context(tc.tile_pool(name="sbt", bufs=4))
        work = ctx.enter_context(tc.tile_pool(name="work", bufs=1))
        small = ctx.enter_context(tc.tile_pool(name="small", bufs=2))
        tpsum = ctx.enter_context(tc.tile_pool(name="tpsum", bufs=2, space="PSUM"))
        rpsum = ctx.enter_context(tc.tile_pool(name="rpsum", bufs=2, space="PSUM"))
        spsum = ctx.enter_context(tc.tile_pool(name="spsum", bufs=4, space="PSUM"))

        # ---- phase A: every DMA trigger up front so HBM saturates from
        # the first microsecond. Tiles stream in pair order: q0 k0 q1 k1.
        # u first: it is tiny and everything's init depends on it; queued
        # later it would sit behind 1MiB chunk transfers.
        u = work.tile([128, 2, BUCKETS], fp32, tag="u")
        nc.sync.dma_start(out=u[:], in_=guv)
        tiles = [(qv, 0), (kv, 0), (qv, 1), (kv, 1)]
        chunk_tiles = [[None] * N_CHUNKS for _ in tiles]
        for idx, (view, t) in enumerate(tiles):
            for c in range(N_CHUNKS):
                ch = chunks.tile([128, CHUNK_F], fp32, tag="chunk",
                                 name=f"ch{idx}_{c}")
                nc.sync.dma_start(out=ch[:],
                                  in_=view[128 * t:128 * (t + 1), c, :])
                chunk_tiles[idx][c] = ch

        # ---- constants, built on GpSimd inside the context (tracked)
        ident = consts.tile([128, 128], fp32)
        make_identity(nc, ident[:])
        # block-diagonal ones: colsum matmul lhsT; out[m,f] = sum over the
        # 64-partition block containing m -> column sums pre-broadcast.
        blockwide = consts.tile([128, 128], fp32)
        nc.gpsimd.memset(blockwide[:], 0.0)
        nc.gpsimd.memset(blockwide[0:64, 0:64], 1.0)
        nc.gpsimd.memset(blockwide[64:128, 64:128], 1.0)
        epsb = consts.tile([128, 1], fp32)
        nc.gpsimd.memset(epsb[:], EPS)

        # gumbel preprocessing: u2 = ln(-ln(u+eps)+eps)
        nc.scalar.activation(out=u[:], in_=u[:], func=AF.Ln, bias=epsb[:])
        nc.scalar.activation(out=u[:], in_=u[:], func=AF.Ln, bias=epsb[:],
                             scale=-1.0)

        # ---- within-bucket fold, split per chunk across engines:
        # GpSimd does pass 1 (one contiguous 1024-elem halving add, the
        # only size where its per-op overhead amortizes), DVE finishes
        # with a single strided reduce into the chunk's partial slot.
        part_tiles = [parts.tile([128, N_CHUNKS, D], fp32, tag=f"part{i}",
                                 name=f"part{i}") for i in range(4)]
        HALF = CHUNK_F // 2
        for idx in (0, 1, 3):         # GpSimd: pass-1 where it is free in time
            for c in range(N_CHUNKS):
                ch = chunk_tiles[idx][c]
                nc.gpsimd.tensor_add(ch[:, 0:HALF], ch[:, 0:HALF],
                                     ch[:, HALF:CHUNK_F])

        def fold_finish(idx, c):      # DVE: [d][16w] strided reduce
            ch = chunk_tiles[idx][c]
            if idx == 2:              # q1 folds fully on DVE (GpSimd busy)
                nc.vector.tensor_add(ch[:, 0:HALF], ch[:, 0:HALF],
                                     ch[:, HALF:CHUNK_F])
            nc.vector.reduce_sum(
                out=part_tiles[idx][:, c, :],
                in_=ch[:, 0:HALF].rearrange("p (w d) -> p d w", d=D),
                axis=AX.X,
            )

        def sums_T(part, tag):
            """Combine chunk partials, return (64=d, 128=rows) SBUF."""
            s = sums.tile([128, D], fp32, tag="sums", name=f"s{tag}")
            nc.vector.reduce_sum(
                out=s[:], in_=part[:].rearrange("p c d -> p d c"), axis=AX.X
            )
            tp = tpsum.tile([64, 128], fp32, tag="tp", name=f"tp{tag}")
            nc.tensor.transpose(tp[:], s[:], ident[:])
            st = sbt.tile([64, 128], fp32, tag=f"T{tag}", name=f"st{tag}")
            nc.scalar.copy(st[:], tp[:])
            return st

        def pair_init(g):
            """R matmuls + gumbel init; returns (p, rsum0)."""
            qT = sums_T(part_tiles[2 * g], f"q{g}")
            kT = sums_T(part_tiles[2 * g + 1], f"k{g}")
            rp = rpsum.tile([128, BUCKETS], fp32, tag="rp", name=f"rp{g}")
            for h in range(2):
                nc.tensor.matmul(
                    rp[64 * h:64 * (h + 1), :],
                    qT[:, 64 * h:64 * (h + 1)],
                    kT[:, 64 * h:64 * (h + 1)],
                    start=True, stop=True,
                    tile_position=(0, 64 * h),
                )
            p = work.tile([128, BUCKETS], fp32, tag=f"p{g}", name=f"p{g}")
            # p = relu(R*scale); r0 = ln(p+eps); p = exp((r0-u2)/T), rowsums
            nc.scalar.activation(out=p[:], in_=rp[:], func=AF.Relu, scale=R_SCALE)
            nc.scalar.activation(out=p[:], in_=p[:], func=AF.Ln, bias=epsb[:])
            nc.vector.tensor_sub(p[:], p[:], u[:, g, :])
            rsum = small.tile([128, 1], fp32, tag=f"rsum{g}", name=f"rs{g}")
            nc.scalar.activation(out=p[:], in_=p[:], func=AF.Exp,
                                 scale=1.0 / TEMP, accum_out=rsum[:])
            return p, rsum

        def sinkhorn_iter(g, it, p, rsum):
            """One row+col normalization; returns next-iteration rowsums."""
            rrec = small.tile([128, 1], fp32, tag=f"rrec{g}", name=f"rr{g}_{it}")
            nc.vector.reciprocal_approx_fast(rrec[:], rsum[:])
            nc.vector.tensor_scalar_mul(p[:], p[:], rrec[:])
            cs = spsum.tile([128, BUCKETS], fp32, tag="cs", name=f"cs{g}_{it}")
            nc.tensor.matmul(cs[:], blockwide[:], p[:], start=True, stop=True)
            cr = small.tile([128, BUCKETS], fp32, tag=f"cr{g}", name=f"cr{g}_{it}")
            nc.vector.reciprocal_approx_fast(cr[:], cs[:])
            nc.vector.tensor_mul(p[:], p[:], cr[:])
            if it == SINKHORN_ITER - 1:
                return None           # final row-sums are never consumed
            nrs = small.tile([128, 1], fp32, tag=f"rsum{g}", name=f"rs{g}_{it}")
            nc.vector.reduce_sum(out=nrs[:], in_=p[:], axis=AX.X)
            return nrs

        # All folds in arrival order, then both pairs' Sinkhorn chains run
        # concurrently with iterations interleaved: each chain's PE/sem
        # gaps are filled by the other chain's DVE work.
        for idx in range(2):
            for c in range(N_CHUNKS):
                fold_finish(idx, c)
        p0, rs0 = pair_init(0)
        for idx in range(2, 4):
            for c in range(N_CHUNKS):
                fold_finish(idx, c)
        p1, rs1 = pair_init(1)

        for it in range(SINKHORN_ITER):
            rs0 = sinkhorn_iter(0, it, p0, rs0)
            rs1 = sinkhorn_iter(1, it, p1, rs1)
        nc.sync.dma_start(out=outv[:, 0, :], in_=p0[:])
        nc.sync.dma_start(out=outv[:, 1, :], in_=p1[:])

    return nc


def _get_nc():
    global _NC_CACHE
    if _NC_CACHE is None:
        _NC_CACHE = _build()
        # Bacc legalization (sync-wait splitting, register allocation)
        # runs in finalize(); the PJRT exec path serializes nc as-is.
        if not _NC_CACHE.is_finalized():
            _NC_CACHE.finalize()
    return _NC_CACHE


def _shard(q, k, gumbel_u):
    return [
        {
            "q": np.ascontiguousarray(q[B_PER * c:B_PER * (c + 1)]),
            "k": np.ascontiguousarray(k[B_PER * c:B_PER * (c + 1)]),
            "gumbel_u": np.ascontiguousarray(gumbel_u[B_PER * c:B_PER * (c + 1)]),
        }
        for c in range(N_CORES)
    ]


def kernel(q, k, gumbel_u, **_unused):
    from concourse.bass_utils import run_bass_kernel_spmd

    q = np.asarray(q, dtype=np.float32)
    k = np.asarray(k, dtype=np.float32)
    gumbel_u = np.asarray(gumbel_u, dtype=np.float32)

    nc = _get_nc()
    res = run_bass_kernel_spmd(nc, _shard(q, k, gumbel_u),
                               core_ids=list(range(N_CORES)))
    return np.concatenate([r["out"] for r in res.results], axis=0)

